# revision 29
# baseline (speedup 1.0000x reference)
"""Trainium2 Bass kernel for nn_MixedAttnHeadEmbed_82076825027210.

Computes, per batch element:
    out = sum over h in {4, 8, 12} of CausalAttention(Q_mix_h, K_mix_h, V_mix_h)
where Q/K/V_mix_h are weighted mixtures (9 scalar weights) of head-sliced
views of x's q/k/v channel groups, zero-padded per head to hd_max = 768/h.

Sharding: data-parallel over batch B=8 across the 8 NeuronCores (one batch
element per core); the 9 mixture weights are baked into the compiled program
as immediates.

Per-core plan (T=1024 tokens, bf16 compute):
  1. SWDGE cast-DMAs load x f32 -> SBUF bf16, split per tensor/half so the
     first config's Q mixing starts ~3.5us in.
  2. DVE mixes Q/K (configs h=4,8; h=12 mixes on Pool to balance load) into
     natural-layout half tiles; Pool mixes V_aug per half (with a
     ones-column per head for the softmax denominator).
  3. Q^T/K^T [d, 512] operands per half: config 0's first half transposes on
     PE (+ACT eviction) since DMA is busy with loads then; everything else
     bounces through DRAM (one write per tensor-half) and returns via HWDGE
     DMA-transpose.
  4. Per config, in passes of 2-4 heads: S^T = K_mix Q_mix^T blockwise on PE
     (causal blocks only; diagonal masked per head by a matmul against a
     strict-triangular constant), exp on ACT (softmax scale folded in;
     max-subtraction skipped since |S*scale| is small), then Y^T = V_aug^T P^T
     accumulated in PSUM over tk (the ones-column yields the softmax
     denominator l as an extra row of Y^T).
  5. Y^T -> SBUF (DVE), PE-transpose all 4 q-tiles of a half into one PSUM
     tile, copy once to SBUF (frees the PSUM ring fast), normalize by 1/l
     and sum across configs into per-half bf16 accumulators; two SWDGE
     cast-DMAs write the f32 result.
"""

import math

import numpy as np

import concourse.bass as bass
import concourse.bacc as bacc
import concourse.tile as tile
from concourse import mybir
from concourse.bass_utils import run_bass_kernel_spmd
from concourse.tile import add_dep_helper

F32 = mybir.dt.float32
BF16 = mybir.dt.bfloat16
ALU = mybir.AluOpType
ACTF = mybir.ActivationFunctionType

T = 1024
NT = 8  # token tiles of 128
E = 768
CIN = 3 * E
N_HEAD_LIST = (4, 8, 12)
EMBED_DIM_LIST = (384, 576, 768)
N_CORES = 8
MASK_NEG = -3000.0  # pre-scale additive mask; exp(scale*-3000) == 0 in f32


def _qtw(h):
    """Transposed-layout total rows: h=8 pads each 96-d head to 128 rows so
    every matmul operand slice starts at a legal base partition (0/32/64)."""
    return 1024 if h == 8 else E


def _dchunks(h):
    """Per head: contraction (d) ranges split at 128-row QT tile boundaries."""
    hd = E // h
    stride = _qtw(h) // h
    out = []
    for i in range(h):
        a, b = i * stride, i * stride + hd
        chunks = []
        while a < b:
            nxt = min(b, (a // 128 + 1) * 128)
            chunks.append((a, nxt))
            a = nxt
        out.append(chunks)
    return out


def _passes(h):
    if h == 4:
        return [[0, 1], [2, 3]]
    if h == 8:
        return [[0, 1, 2, 3], [4, 5, 6, 7]]
    return [[0, 1, 2, 3], [4, 5, 6, 7], [8, 9, 10, 11]]


def _vchunks(h):
    """Column ranges of one head's (hd+1)-wide V_aug block, <=128 rows each."""
    hd = E // h
    if hd + 1 > 128:
        return [(0, 128), (128, hd + 1)]
    return [(0, hd + 1)]


def _build_program(W):
    """W: numpy [9] f32 mixture weights. Returns compiled Bacc program."""
    nc = bacc.Bacc(
        "TRN2", target_bir_lowering=False, debug=False, num_devices=N_CORES
    )
    x_in = nc.dram_tensor("x", [T, CIN], F32, kind="ExternalInput").ap()
    out_d = nc.dram_tensor("out", [T, E], F32, kind="ExternalOutput").ap()
    qk_dram = [
        [
            nc.dram_tensor(
                f"qkb_{ci}_{ti}", [T, _qtw(N_HEAD_LIST[ci])], BF16
            ).ap()
            for ti in range(2)
        ]
        for ci in range(3)
    ]

    with tile.TileContext(nc) as tc:
        _emit(tc, x_in, out_d, qk_dram, W)
    nc.compile()
    return nc


def _emit(tc, x_in, out_d, qk_dram, W):
    nc = tc.nc
    with (
        tc.tile_pool(name="consts", bufs=1) as consts,
        tc.tile_pool(name="xload", bufs=1) as xload_pool,
        tc.tile_pool(name="natc", bufs=2) as natc_pool,
        tc.tile_pool(name="vtmp", bufs=2) as vtmp_pool,
        tc.tile_pool(name="qktmp", bufs=1) as qktmp_pool,
        tc.tile_pool(name="vaug", bufs=4) as vaug_pool,
        tc.tile_pool(name="qt", bufs=8) as qt_pool,
        tc.tile_pool(name="pt", bufs=4) as pt_pool,
        tc.tile_pool(name="ytsb", bufs=3) as ytsb_pool,
        tc.tile_pool(name="ynsb", bufs=2) as ynsb_pool,
        tc.tile_pool(name="small", bufs=4) as small_pool,
        tc.tile_pool(name="oacc", bufs=4) as oacc_pool,
        tc.tile_pool(name="stage", bufs=2, space="PSUM") as stage_pool,
        tc.tile_pool(name="ypsum", bufs=2, space="PSUM") as ypsum_pool,
    ):
        # ---- constants -------------------------------------------------
        ident = consts.tile([128, 128], BF16)
        nc.gpsimd.memset(ident, 0.0)
        nc.gpsimd.affine_select(
            out=ident, in_=ident, compare_op=ALU.not_equal, fill=1.0,
            base=0, pattern=[[-1, 128]], channel_multiplier=1,
        )
        # ustrict[d, t] = 1 if t > d else 0
        ustrict = consts.tile([128, 128], BF16)
        nc.gpsimd.memset(ustrict, 1.0)
        nc.gpsimd.affine_select(
            out=ustrict, in_=ustrict, compare_op=ALU.is_gt, fill=0.0,
            base=0, pattern=[[1, 128]], channel_multiplier=-1,
        )
        # negi = MASK_NEG * I
        negi = consts.tile([128, 128], BF16)
        nc.gpsimd.memset(negi, 0.0)
        nc.gpsimd.affine_select(
            out=negi, in_=negi, compare_op=ALU.not_equal, fill=MASK_NEG,
            base=0, pattern=[[-1, 128]], channel_multiplier=1,
        )

        # ---- x loads: q-h0 and k-h0 split so Q mixing starts early -----
        xq0 = xload_pool.tile([128, 4, E], BF16, tag="xq0", name="xq0")
        xk0 = xload_pool.tile([128, 4, E], BF16, tag="xk0", name="xk0")
        xqk1 = xload_pool.tile([128, 4, 2 * E], BF16, tag="xqk1",
                               name="xqk1")
        xv = [xload_pool.tile([128, 4, E], BF16, tag=f"xv{hf}",
                              name=f"xv{hf}") for hf in range(2)]
        h0 = slice(0, 512)
        h1 = slice(512, 1024)
        nc.gpsimd.dma_start(
            out=xq0[:, :, :],
            in_=x_in[h0, 0:E].rearrange("(a p) c -> p a c", p=128),
        )
        nc.gpsimd.dma_start(
            out=xk0[:, :, :],
            in_=x_in[h0, E : 2 * E].rearrange("(a p) c -> p a c", p=128),
        )
        nc.gpsimd.dma_start(
            out=xqk1[:, :, :],
            in_=x_in[h1, 0 : 2 * E].rearrange("(a p) c -> p a c", p=128),
        )
        nc.gpsimd.dma_start(
            out=xv[0][:, :, :],
            in_=x_in[h0, 2 * E :].rearrange("(a p) c -> p a c", p=128),
        )
        nc.gpsimd.dma_start(
            out=xv[1][:, :, :],
            in_=x_in[h1, 2 * E :].rearrange("(a p) c -> p a c", p=128),
        )

        # quarter-granularity accumulators: [s*2 + tt//2] -> rows 256*q
        oaccs = [
            oacc_pool.tile([128, 2, E], BF16, tag="oacc", name=f"oacc{q}")
            for q in range(4)
        ]

        # weight order in W: for cfg ci, e in (384, 576, 768): W[3*ci + idx]
        for ci, h in enumerate(N_HEAD_LIST):
            hd = E // h
            scale = 1.0 / math.sqrt(hd)
            dchunks = _dchunks(h)
            vchunks = _vchunks(h)
            pw = _qtw(h) // h
            qk_on_pool = ci == 2  # h=12 mixes Q/K on Pool

            nats = [
                natc_pool.tile(
                    [128, 4, 2, h, pw], BF16, tag="natc", name=f"natc{ci}{hf}"
                )
                for hf in range(2)
            ]
            for hf in range(2):
                if pw > hd:
                    nc.vector.memset(nats[hf][:, :, :, :, hd:pw], 0.0)

            # ---- Q/K mixing --------------------------------------------
            # half 0: separate per-tensor ops (q/k live in separate tiles)
            for ti, xt in ((0, xq0), (1, xk0)):
                for tt in range(4):
                    def oap(e, tt=tt, ti=ti):
                        return nats[0][:, tt, ti, :, 0 : e // h]

                    def iap(e, xt=xt, tt=tt):
                        return xt[:, tt, 0:e].rearrange(
                            "p (h d) -> p h d", h=h
                        )

                    if qk_on_pool:
                        nc.gpsimd.tensor_scalar(
                            oap(768), iap(768), float(W[3 * ci + 2]), None,
                            ALU.mult,
                        )
                        for e, wi in ((576, 1), (384, 0)):
                            tmp = qktmp_pool.tile(
                                [128, h, 576 // h], BF16, tag="qktmp"
                            )
                            tv = tmp[:, :, 0 : e // h]
                            nc.gpsimd.tensor_scalar(
                                tv, iap(e), float(W[3 * ci + wi]), None,
                                ALU.mult,
                            )
                            nc.gpsimd.tensor_tensor(
                                oap(e), tv, oap(e), ALU.add
                            )
                    else:
                        nc.vector.tensor_scalar(
                            oap(768), iap(768), float(W[3 * ci + 2]), None,
                            ALU.mult,
                        )
                        for e, wi in ((576, 1), (384, 0)):
                            nc.vector.scalar_tensor_tensor(
                                out=oap(e), in0=iap(e),
                                scalar=float(W[3 * ci + wi]), in1=oap(e),
                                op0=ALU.mult, op1=ALU.add,
                            )
            # half 1: fused two-tensor ops from xqk1
            for tt in range(4):
                xqk_t = xqk1[:, tt, :].rearrange("p (qk c) -> p qk c", qk=2)

                def oap(e, tt=tt):
                    return nats[1][:, tt, :, :, 0 : e // h]

                def iap(e, xqk_t=xqk_t):
                    return xqk_t[:, :, 0:e].rearrange(
                        "p qk (h d) -> p qk h d", h=h
                    )

                if qk_on_pool:
                    nc.gpsimd.tensor_scalar(
                        oap(768), iap(768), float(W[3 * ci + 2]), None,
                        ALU.mult,
                    )
                    for e, wi in ((576, 1), (384, 0)):
                        tmp = qktmp_pool.tile(
                            [128, 2, h, 576 // h], BF16, tag="qktmp2"
                        )
                        tv = tmp[:, :, :, 0 : e // h]
                        nc.gpsimd.tensor_scalar(
                            tv, iap(e), float(W[3 * ci + wi]), None, ALU.mult
                        )
                        nc.gpsimd.tensor_tensor(oap(e), tv, oap(e), ALU.add)
                else:
                    nc.vector.tensor_scalar(
                        oap(768), iap(768), float(W[3 * ci + 2]), None,
                        ALU.mult,
                    )
                    for e, wi in ((576, 1), (384, 0)):
                        nc.vector.scalar_tensor_tensor(
                            out=oap(e), in0=iap(e),
                            scalar=float(W[3 * ci + wi]), in1=oap(e),
                            op0=ALU.mult, op1=ALU.add,
                        )

            # ---- V_aug mixing on Pool, per half ------------------------
            vaugs = []
            for hf in range(2):
                va = vaug_pool.tile(
                    [128, 4, h, hd + 1], BF16, tag="vaug",
                    name=f"vaug{ci}{hf}",
                )
                nc.vector.memset(va[:, :, :, hd : hd + 1], 1.0)
                for tt in range(4):
                    def oap(e, tt=tt, va=va):
                        return va[:, tt, :, 0 : e // h]

                    def iap(e, hf=hf, tt=tt):
                        return xv[hf][:, tt, 0:e].rearrange(
                            "p (h d) -> p h d", h=h
                        )

                    nc.gpsimd.tensor_scalar(
                        oap(768), iap(768), float(W[3 * ci + 2]), None,
                        ALU.mult,
                    )
                    for e, wi in ((576, 1), (384, 0)):
                        tmp = vtmp_pool.tile(
                            [128, h, 576 // h], BF16, tag="vtmp"
                        )
                        tv = tmp[:, :, 0 : e // h]
                        nc.gpsimd.tensor_scalar(
                            tv, iap(e), float(W[3 * ci + wi]), None, ALU.mult
                        )
                        nc.gpsimd.tensor_tensor(oap(e), tv, oap(e), ALU.add)
                vaugs.append(va)

            # ---- Q^T/K^T operands --------------------------------------
            ndt = _qtw(h) // 128
            qth = [[None, None], [None, None]]  # [ti][hf]
            for ti in range(2):
                for hf in range(2):
                    qth[ti][hf] = qt_pool.tile(
                        [128, ndt, 512], BF16, tag="qt",
                        name=f"qt{ci}{ti}{hf}",
                    )
            if ci == 0:
                # first half via PE transpose + ACT eviction (DMA engines
                # are busy with x loads at kernel start)
                for ti in range(2):
                    flat = nats[0][:, :, ti, :, :].rearrange(
                        "p t h w -> p t (h w)"
                    )
                    for cb in range(ndt):
                        ps = ypsum_pool.tile([128, 4, 128], BF16, tag="y",
                                             name="tps")
                        for tt in range(4):
                            nc.tensor.transpose(
                                out=ps[:, tt, :],
                                in_=flat[:, tt, cb * 128 : (cb + 1) * 128],
                                identity=ident[:, :],
                            )
                        nc.scalar.copy(
                            qth[ti][0][:, cb, :],
                            ps.rearrange("p a b -> p (a b)"),
                        )
                dma_halves = (1,)
            else:
                dma_halves = (0, 1)
            for hf in dma_halves:
                rows = slice(hf * 512, hf * 512 + 512)
                for ti in range(2):
                    wins = nc.sync.dma_start(
                        out=qk_dram[ci][ti][rows, :].rearrange(
                            "(a p) c -> p a c", p=128
                        ),
                        in_=nats[hf][:, :, ti, :, :],
                    )
                    tl = qth[ti][hf]
                    for dt_ in range(ndt):
                        ins = nc.sync.dma_start(
                            out=tl[:, dt_, :],
                            in_=qk_dram[ci][ti][
                                rows, dt_ * 128 : (dt_ + 1) * 128
                            ],
                            transpose=True,
                        )
                        add_dep_helper(
                            ins.ins, wins.ins, reason="dram bounce raw"
                        )
            qt, kt = qth

            # ---- attention for this config ------------------------------
            def run_pass(s, pheads):
                nh = len(pheads)
                ncph = len(vchunks)
                # chunk tiles reordered so psum-pair partners have equal rows:
                # all heads' chunk 0, then all heads' chunk 1, ...
                ctiles = [
                    (hh, c0, c1) for (c0, c1) in vchunks for hh in pheads
                ]
                offs = []
                acc = 0
                for (_, c0, c1) in ctiles:
                    offs.append(acc)
                    acc += c1 - c0
                    acc = (acc + 1) & ~1  # psum bf16 needs 4B-aligned offsets
                # l column (within packed ynat cols) per head position
                lcols = [
                    offs[(ncph - 1) * nh + p] + (hd - vchunks[-1][0])
                    for p in range(nh)
                ]
                lbase = lcols[0]
                lstride = (lcols[1] - lcols[0]) if nh > 1 else 1
                assert all(
                    lcols[p] == lbase + p * lstride for p in range(nh)
                ), lcols
                groups = [pheads[i : i + 2] for i in range(0, nh, 2)]

                ntk = 4 * s + 4
                yts = [
                    ypsum_pool.tile(
                        [128, 2, 512], F32, tag="y", name=f"yt{k}"
                    )
                    for k in range((len(ctiles) + 1) // 2)
                ]

                def emit_pv(tk, lo, pts_tk):
                    for k, (head, c0, c1) in enumerate(ctiles):
                        gi = pheads.index(head) // 2
                        j = pheads.index(head) % 2
                        nc.tensor.matmul(
                            out=yts[k // 2][0 : c1 - c0, k % 2, lo:512],
                            lhsT=vaugs[tk // 4][:, tk % 4, head, c0:c1],
                            rhs=pts_tk[gi][:, j, lo:512],
                            start=(tk == 0),
                            stop=(tk == ntk - 1),
                        )

                prev = None  # (tk, lo, pts) -- PV lags S/exp by one tk
                for tk in range(ntk):
                    lo = max(0, tk * 128 - s * 512)
                    ktile = kt[tk // 4]
                    tkc = (tk % 4) * 128
                    pts = {}
                    for gi, g in enumerate(groups):
                        stage = stage_pool.tile([128, 2, 512], F32)
                        for j, head in enumerate(g):
                            n_mm = len(dchunks[head]) + (
                                1 if tk // 4 == s else 0
                            )
                            mi = 0
                            for (a, b) in dchunks[head]:
                                nc.tensor.matmul(
                                    out=stage[:, j, lo:512],
                                    lhsT=ktile[
                                        a % 128 : a % 128 + (b - a),
                                        a // 128,
                                        tkc : tkc + 128,
                                    ],
                                    rhs=qt[s][
                                        a % 128 : a % 128 + (b - a),
                                        a // 128,
                                        lo:512,
                                    ],
                                    start=(mi == 0),
                                    stop=(mi == n_mm - 1),
                                )
                                mi += 1
                            if tk // 4 == s:
                                # diagonal block: MASK_NEG above diagonal
                                nc.tensor.matmul(
                                    out=stage[:, j, lo : lo + 128],
                                    lhsT=ustrict[:, :],
                                    rhs=negi[:, :],
                                    start=False,
                                    stop=True,
                                )
                        pt = pt_pool.tile([128, 2, 512], BF16, tag="pt")
                        nc.scalar.activation(
                            out=pt[:, 0 : len(g), lo:512],
                            in_=stage[:, 0 : len(g), lo:512],
                            func=ACTF.Exp,
                            scale=scale,
                        )
                        pts[gi] = pt
                        if gi == 0 and prev is not None:
                            emit_pv(*prev)
                            prev = None
                    prev = (tk, lo, pts)
                emit_pv(*prev)
                # ---- evict Y^T (DVE), transpose to natural ----------
                ytsbs = []
                for kp in range(len(yts)):
                    rows = ctiles[2 * kp][2] - ctiles[2 * kp][1]
                    nslots = min(2, len(ctiles) - 2 * kp)
                    sb = ytsb_pool.tile(
                        [128, 2, 512], BF16, tag="ytsb", name=f"ysb{kp}"
                    )
                    nc.vector.tensor_copy(
                        sb[0:rows, 0:nslots, :],
                        yts[kp][0:rows, 0:nslots, :],
                    )
                    ytsbs.append(sb)
                # one psum tile for all 4 q-tiles of this half; copied
                # to SBUF immediately so the psum ring frees fast
                yn4 = ypsum_pool.tile([128, 4, 512], BF16, tag="y",
                                      name="yn4")
                for tt in range(4):
                    for k, (head, c0, c1) in enumerate(ctiles):
                        rows = c1 - c0
                        nc.tensor.transpose(
                            out=yn4[:, tt, offs[k] : offs[k] + rows],
                            in_=ytsbs[k // 2][
                                0:rows, k % 2, tt * 128 : (tt + 1) * 128
                            ],
                            identity=ident[0:rows, 0:rows],
                        )
                # contiguous written runs of the packed yn4 columns
                runs = []
                for k, (_, c0, c1) in enumerate(ctiles):
                    if runs and runs[-1][1] == offs[k]:
                        runs[-1][1] = offs[k] + (c1 - c0)
                    else:
                        runs.append([offs[k], offs[k] + (c1 - c0)])
                ysb = ynsb_pool.tile([128, 4, 512], BF16, tag="ynsb")
                for r0, r1 in runs:
                    nc.vector.tensor_copy(ysb[:, :, r0:r1], yn4[:, :, r0:r1])
                lrow = small_pool.tile([128, 4, 4], F32, tag="lrow")
                rec = small_pool.tile([128, 4, 4], F32, tag="rec")
                nc.vector.tensor_copy(
                    lrow[:, :, 0:nh],
                    ysb[
                        :, :,
                        lbase : lbase + (nh - 1) * lstride + 1 : lstride,
                    ],
                )
                nc.vector.reciprocal(rec[:, :, 0:nh], lrow[:, :, 0:nh])
                for tt in range(4):
                    oacc = oaccs[s * 2 + tt // 2]
                    for k, (head, c0, c1) in enumerate(ctiles):
                        datarows = min(c1, hd) - c0
                        p = pheads.index(head)
                        dst = oacc[
                            :,
                            tt % 2,
                            head * hd + c0 : head * hd + c0 + datarows,
                        ]
                        src = ysb[:, tt, offs[k] : offs[k] + datarows]
                        if ci == 0:
                            nc.vector.tensor_scalar(
                                dst, src, rec[:, tt, p : p + 1], None,
                                ALU.mult,
                            )
                        else:
                            nc.vector.scalar_tensor_tensor(
                                out=dst,
                                in0=src,
                                scalar=rec[:, tt, p : p + 1],
                                in1=dst,
                                op0=ALU.mult,
                                op1=ALU.add,
                            )

            # s outer: all passes' first q-half run before any second-half
            # work, so half-1 operands (DMA transposes, V mixing) have a
            # full half-attention of slack.
            for s_outer in range(2):
                for pheads in _passes(h):
                    run_pass(s_outer, pheads)

        # ---- write result (bf16 -> f32 cast via SWDGE), per quarter -----
        for q in range(4):
            rows = slice(q * 256, q * 256 + 256)
            nc.gpsimd.dma_start(
                out=out_d[rows, :].rearrange("(a p) c -> p a c", p=128),
                in_=oaccs[q][:, :, :],
            )


_PROGRAM_CACHE = {}


def _get_program(W):
    key = np.asarray(W, dtype=np.float32).tobytes()
    if key not in _PROGRAM_CACHE:
        _PROGRAM_CACHE[key] = _build_program(np.asarray(W, dtype=np.float32))
    return _PROGRAM_CACHE[key]


def kernel(x, weights):
    """x: [8, 1024, 2304] f32; weights: [9] f32 -> [8, 1024, 768] f32."""
    x = np.asarray(x, dtype=np.float32)
    weights = np.asarray(weights, dtype=np.float32)
    assert x.shape == (N_CORES, T, CIN), x.shape
    nc = _get_program(weights)
    in_maps = [{"x": np.ascontiguousarray(x[c])} for c in range(N_CORES)]
    res = run_bass_kernel_spmd(nc, in_maps, list(range(N_CORES)))
    return np.stack([res.results[c]["out"] for c in range(N_CORES)], axis=0)
